# revision 1
# baseline (speedup 1.0000x reference)
"""Bidirectional chamfer loss on 8 Trainium2 NeuronCores.

Problem: N=16384 render points (128x128x2), M=16384 contour points (16384x2),
output = sum_i min_j ||p_i - q_j|| + sum_j min_i ||p_i - q_j||  (scalar f32).

Strategy (retrieval_knn):
  - Host: sort p and q by x-coordinate. Core c gets the c-th slice of 2048
    sorted render points plus a window of W=4096 contiguous sorted contour
    points centered on the slice's x-range. For uniform points in a 512px
    image the window gives >=~32px of x-margin on each side, far beyond any
    nearest-neighbor distance, so windowed mins equal true mins. This is
    *certified exactly* on the host afterwards (excluded points are at least
    the window-edge x-distance away); any failing row/column falls back to
    an exact numpy computation, so the kernel is correct for any input.
  - Device (per core): tensor engine computes d2/16 blocks directly via a
    K=4 matmul (lhsT rows [px/4, py/4, p2/16, 1] x rhs rows
    [-2qx/4, -2qy/4, 1, q2/16]); both the (p-rows x q-cols) matrix and its
    transpose are produced so each min direction is a free-axis reduce_min.
    No cross-core communication: row mins are per-slice, column mins are
    per-window and host-combined with a scatter-min.
  - Host: sqrt + sums in float64, cast to float32.
"""

import numpy as np

# ---- hardcoded problem geometry (from the problem spec) ----
N = 16384            # render points (128*128)
M = 16384            # contour points
NCORES = 8
NP_CORE = N // NCORES          # 2048 render points per core
W = 4096                       # contour window per core
P = 128                        # partitions
IT_A = NP_CORE // P            # 16 i-tiles (matrix A)
IT_B = W // P                  # 32 j-tiles (matrix B)
CHUNK = 2048                   # psum chunk free size (4 banks)
MMF = 512                      # fp32 matmul max moving free dim

_COMPILED = {}


def _build_program():
    """Build the SPMD bass program (same program for all 8 cores).

    Raw bass (not Tile): the pipeline is a simple PE->DVE double buffer and
    Tile's semaphore pass emits 2 waits on the first matmul of a reused PSUM
    slot, which walrus can't encode (1 wait slot per instruction). With
    explicit Block bodies every wait is a standalone instruction.
    """
    import concourse.bass as bass
    from concourse import mybir

    f32 = mybir.dt.float32
    X = mybir.AxisListType.X
    MIN = mybir.AluOpType.min

    nc = bass.Bass("TRN2", target_bir_lowering=False, debug=False,
                   num_devices=NCORES)

    TOT = NP_CORE + W + W + NP_CORE   # 12288
    inp = nc.dram_tensor("inp", [4, TOT], f32, kind="ExternalInput").ap()
    rowout = nc.dram_tensor("rowout", [P, IT_A], f32, kind="ExternalOutput").ap()
    colout = nc.dram_tensor("colout", [P, IT_B], f32, kind="ExternalOutput").ap()

    o0, o1, o2 = NP_CORE, NP_CORE + W, NP_CORE + 2 * W
    NCH_A = IT_A * (W // CHUNK)       # 32 chunks for matrix A
    NCH_B = IT_B                      # 32 chunks for matrix B (CHUNK == NP_CORE)
    NCH = NCH_A + NCH_B

    with (
        nc.sbuf_tensor([128, TOT], f32) as t_inp,
        nc.sbuf_tensor([P, NCH_A], f32) as accA,
        nc.sbuf_tensor([P, NCH_B], f32) as accB,
        nc.sbuf_tensor([P, IT_A], f32) as rmin,
        nc.psum_tensor([P, CHUNK], f32) as ps0,
        nc.psum_tensor([P, CHUNK], f32) as ps1,
        nc.semaphore() as dma_sem,
        nc.semaphore() as pe_sem,
        nc.semaphore() as dve_sem,
        nc.Block() as block,
    ):
        def chunk_aps(k):
            """([4 lhsT APs], [4 rhs APs], accum AP) for chunk k.

            The m-th matmul of a chunk reads its K=4 operands from the
            replica at partitions 32m..32m+3 and runs in PE row group 32m,
            so all 4 matmuls of a chunk execute concurrently in the array.
            """
            if k < NCH_A:
                t, h = divmod(k, W // CHUNK)
                lc = slice(t * P, (t + 1) * P)
                rc = [slice(o0 + h * CHUNK + m * MMF, o0 + h * CHUNK + (m + 1) * MMF)
                      for m in range(CHUNK // MMF)]
                acc = accA[:, k:k + 1]
            else:
                u = k - NCH_A
                lc = slice(o1 + u * P, o1 + (u + 1) * P)
                rc = [slice(o2 + m * MMF, o2 + (m + 1) * MMF)
                      for m in range(CHUNK // MMF)]
                acc = accB[:, u:u + 1]
            lhsT = [t_inp[32 * m:32 * m + 4, lc] for m in range(CHUNK // MMF)]
            rhs = [t_inp[32 * m:32 * m + 4, rc[m]] for m in range(CHUNK // MMF)]
            return lhsT, rhs, acc

        @block.sync
        def _(sync):
            # replicate the [4, TOT] operand block into PE row groups
            # 0/32/64/96 so K=4 matmuls can pack 4-wide in the array
            for r in (0, 32, 64, 96):
                sync.dma_start(t_inp[r:r + 4, :], inp).then_inc(dma_sem, 16)
            sync.wait_ge(dve_sem, NCH + 1)
            sync.dma_start(rowout, rmin[:]).then_inc(dma_sem, 16)
            sync.dma_start(colout, accB[:]).then_inc(dma_sem, 16)

        @block.tensor
        def _(pe):
            pe.wait_ge(dma_sem, 64)
            for k in range(NCH):
                ps = ps0 if k % 2 == 0 else ps1
                if k >= 2:
                    pe.wait_ge(dve_sem, k - 1)  # slot's previous reduce done
                lhsT, rhs, _ = chunk_aps(k)
                last = None
                for m in range(CHUNK // MMF):
                    last = nc.tensor.matmul(
                        ps[:, m * MMF:(m + 1) * MMF], lhsT[m], rhs[m],
                        start=True, stop=True,
                        tile_position=(32 * m, 0),
                    )
                last.then_inc(pe_sem, 1)

        @block.vector
        def _(vector):
            for k in range(NCH):
                ps = ps0 if k % 2 == 0 else ps1
                vector.wait_ge(pe_sem, k + 1)
                _, _, acc = chunk_aps(k)
                nc.vector.tensor_reduce(
                    acc, ps[:], axis=X, op=MIN,
                ).then_inc(dve_sem, 1)
            # combine the window-halves of each i-tile: [P, 16, 2] -> [P, 16]
            nc.vector.tensor_reduce(
                rmin[:], accA[:].rearrange("p (t h) -> p t h", h=2),
                axis=X, op=MIN,
            ).then_inc(dve_sem, 1)

    return nc


def _get_program():
    if "nc" not in _COMPILED:
        _COMPILED["nc"] = _build_program()
    return _COMPILED["nc"]


def _prep(points, scale_sq):
    """rows [x/4, y/4, sq/16-or-1, 1-or-sq/16] for the K=4 matmul.

    Coordinates deliberately NOT centered: keeping the same term magnitudes
    as the reference's p2+q2-2pq makes our fp32 rounding errors correlate
    with the reference's, minimizing the deviation from its fp32 output
    (measured 7.4e-5 uncentered vs 9.8e-5 centered).
    """
    x = points[:, 0].astype(np.float32)
    y = points[:, 1].astype(np.float32)
    sq = (x * x + y * y) / np.float32(16.0)
    ones = np.ones_like(x)
    if scale_sq == "lhsT":   # stationary side: [x/4, y/4, sq/16, 1]
        return np.stack([x / 4.0, y / 4.0, sq, ones]).astype(np.float32)
    else:                     # moving side: [-2x/4, -2y/4, 1, sq/16]
        return np.stack([-x / 2.0, -y / 2.0, ones, sq]).astype(np.float32)


def _make_in_maps(p: np.ndarray, q: np.ndarray):
    """Sort by x, slice/window per core, build device operands."""
    po = np.argsort(p[:, 0], kind="stable")
    qo = np.argsort(q[:, 0], kind="stable")
    ps = p[po]
    qs = q[qo]
    qx = qs[:, 0]

    in_maps = []
    starts = []
    for c in range(NCORES):
        sl = ps[c * NP_CORE:(c + 1) * NP_CORE]
        s_lo = np.searchsorted(qx, sl[0, 0])
        s_hi = np.searchsorted(qx, sl[-1, 0])
        start = int(np.clip((s_lo + s_hi) // 2 - W // 2, 0, M - W))
        starts.append(start)
        qw = qs[start:start + W]
        inp = np.concatenate([
            _prep(sl, "lhsT"), _prep(qw, "rhs"),
            _prep(qw, "lhsT"), _prep(sl, "rhs"),
        ], axis=1)
        in_maps.append({"inp": np.ascontiguousarray(inp)})
    return in_maps, starts, ps, qs


def kernel(img_render_points: np.ndarray, contour_points: np.ndarray) -> np.ndarray:
    # NOTE: do not enable jax_compilation_cache_dir here — loading this
    # program from the jax persistent cache produces executables that fail
    # with NRT_EXEC_UNIT_UNRECOVERABLE on the axon PJRT path. The NEFF
    # compile itself is cached by the environment's own compile cache.
    from concourse.bass_utils import run_bass_kernel_spmd

    p = np.asarray(img_render_points, dtype=np.float32).reshape(-1, 2)
    q = np.asarray(contour_points, dtype=np.float32)
    assert p.shape == (N, 2) and q.shape == (M, 2)

    in_maps, starts, ps, qs = _make_in_maps(p, q)
    qx = qs[:, 0]

    nc = _get_program()
    res = run_bass_kernel_spmd(nc, in_maps, list(range(NCORES)))
    results = res.results

    # ---- host combine ----
    rowmin2 = np.empty(N, dtype=np.float64)   # d2, sorted-p order
    colmin2 = np.full(M, np.inf, dtype=np.float64)  # d2, sorted-q order
    for c in range(NCORES):
        ro = np.asarray(results[c]["rowout"], dtype=np.float64) * 16.0  # [P, IT_A]
        co = np.asarray(results[c]["colout"], dtype=np.float64) * 16.0  # [P, IT_B]
        # rowout[p, t] -> sorted index c*NP_CORE + t*P + p
        rowmin2[c * NP_CORE:(c + 1) * NP_CORE] = ro.T.reshape(-1)
        # colout[p, u] -> window-local j = u*P + p
        w = co.T.reshape(-1)
        seg = slice(starts[c], starts[c] + W)
        np.minimum.at(colmin2, seg, w)

    # ---- exact certification of the windowing ----
    px = ps[:, 0].astype(np.float64)
    qxd = qx.astype(np.float64)
    # rows: excluded contour points are beyond the window edges in x
    row_bound = np.full(N, np.inf)
    for c in range(NCORES):
        s = starts[c]
        idx = slice(c * NP_CORE, (c + 1) * NP_CORE)
        b = np.full(NP_CORE, np.inf)
        if s > 0:
            b = np.minimum(b, np.maximum(px[idx] - qxd[s - 1], 0.0) ** 2)
        if s + W < M:
            b = np.minimum(b, np.maximum(qxd[s + W] - px[idx], 0.0) ** 2)
        row_bound[idx] = b
    bad_rows = np.nonzero(rowmin2 > row_bound)[0]

    # cols: for each contour point, cores that excluded it are at least
    # the x-distance to that core's p-slice away
    col_bound = np.full(M, np.inf)
    for c in range(NCORES):
        s = starts[c]
        pmin = px[c * NP_CORE]
        pmax = px[(c + 1) * NP_CORE - 1]
        d = np.maximum(np.maximum(pmin - qxd, qxd - pmax), 0.0) ** 2
        excl = np.ones(M, dtype=bool)
        excl[s:s + W] = False
        col_bound[excl] = np.minimum(col_bound[excl], d[excl])
    bad_cols = np.nonzero(colmin2 > col_bound)[0]

    # ---- exact numpy fallback for any uncertified entries ----
    if bad_rows.size:
        pp = ps[bad_rows].astype(np.float64)
        qq = qs.astype(np.float64)
        d2 = ((pp[:, None, :] - qq[None, :, :]) ** 2).sum(-1)
        rowmin2[bad_rows] = d2.min(axis=1)
    if bad_cols.size:
        qq = qs[bad_cols].astype(np.float64)
        pp = ps.astype(np.float64)
        d2 = ((qq[:, None, :] - pp[None, :, :]) ** 2).sum(-1)
        colmin2[bad_cols] = d2.min(axis=1)

    total = (np.sqrt(np.maximum(rowmin2, 0.0)).sum()
             + np.sqrt(np.maximum(colmin2, 0.0)).sum())
    return np.float32(total)



# revision 2
# speedup vs baseline: 5.9794x; 5.9794x over previous
"""Bidirectional chamfer loss on 8 Trainium2 NeuronCores.

Problem: N=16384 render points (128x128x2), M=16384 contour points (16384x2),
output = sum_i min_j ||p_i - q_j|| + sum_j min_i ||p_i - q_j||  (scalar f32).

Strategy (retrieval_knn, v2):
  - Host: 2D-tile both point sets (16 x-strips x 8 y-tiles = 128 tiles of 128
    points each side). For each query tile, gather the opposite-side points
    inside the tile's bbox dilated by MARGIN=8px (~240 expected for uniform
    data; W=384 budget, padded by repeating a real candidate so the min is
    unchanged). Points outside the dilated box are >MARGIN away from every
    query in the tile, so the windowed min equals the true min whenever the
    windowed min^2 < MARGIN^2 - slack; this is certified per query on the
    host, with an exact numpy fallback for any failure, so the kernel is
    correct for any input.
  - Device (per core, 16 p-tiles + 16 q-tiles = 32 tiles): the tensor engine
    computes each tile's [128 x 384] d2 block with a K=12 bf16 matmul.
    Coordinates are centered per tile and split hi/lo into bf16 pairs
    (Dekker-style), making the bf16 matmul accurate to ~0.05px^2 while
    running at 1 cycle/row (fp32 is 4). Tiles round-robin the 4 PE row
    quadrants so 4 matmuls pack concurrently in the array.
  - Reduce: the d2 min over candidates. DVE reads PSUM at 1 elem/cycle/lane
    (single PSUM read port), so chunks alternate two paths: (A) DVE
    segmented reduce_min straight from PSUM; (B) scalar-engine copy
    PSUM->SBUF casting to bf16, then DVE reduce_min from SBUF which runs in
    4x packed mode. The two engines stream concurrently.
  - Host: certify, sqrt + sum in float64, cast to float32.
"""

import numpy as np

# ---- hardcoded problem geometry (from the problem spec) ----
N = 16384            # render points (128*128)
M = 16384            # contour points
NCORES = 8
P = 128              # partitions / tile query count
NSTRIP = 16          # x-strips per side
TPS = 8              # y-tiles per strip
NTILE = NSTRIP * TPS              # 128 tiles per side
TPC = NTILE // NCORES             # 16 tiles per side per core
NDT = 2 * TPC                     # 32 device tiles per core
W = 384              # candidate window per tile
MARGIN = 8.0         # bbox dilation in px
K = 12               # matmul contraction rows (hi/lo split form)
NCHUNK = NDT // 4    # 8 chunks of 4 quadrant-packed tiles
CHUNKS_B = (1, 2, 4, 5, 7)        # chunks reduced via ACT-copy + 4x DVE
# per-tile free span in sbuf: 128 lhsT cols + 384 rhs cols
SPAN = P + W         # 512

_COMPILED = {}


def _build_program():
    """Build the SPMD bass program (same program for all 8 cores).

    Raw bass (not Tile): explicit Block bodies keep every semaphore wait a
    standalone instruction (walrus has 1 wait slot per instruction).
    """
    import concourse.bass as bass
    from concourse import mybir

    f32 = mybir.dt.float32
    bf16 = mybir.dt.bfloat16
    X = mybir.AxisListType.X
    MIN = mybir.AluOpType.min
    COPY = mybir.ActivationFunctionType.Copy

    nc = bass.Bass("TRN2", target_bir_lowering=False, debug=False,
                   num_devices=NCORES)

    # dram input: 4 quadrant row-blocks of K=12 rows, 8 tiles x 512 cols
    inp = nc.dram_tensor("inp", [4 * K, 8 * SPAN], bf16,
                         kind="ExternalInput").ap()
    outd = nc.dram_tensor("out", [P, NDT], f32, kind="ExternalOutput").ap()

    chunks_b = set(CHUNKS_B)
    bidx = {k: i for i, k in enumerate(sorted(chunks_b))}   # B-order index

    with (
        nc.sbuf_tensor([128, 8 * SPAN], bf16) as t_inp,
        nc.sbuf_tensor([P, 2, 4 * W], bf16) as bbuf,
        nc.sbuf_tensor([P, NDT], f32) as acc,
        nc.psum_tensor([P, 4096], f32) as ps,
        nc.semaphore() as dma_sem,
        nc.semaphore() as pe_sem,
        nc.semaphore() as act_sem,
        nc.semaphore() as dve_sem,
        nc.Block() as block,
    ):
        def tile_aps(t):
            """(lhsT, rhs, psum_out) APs for device tile t."""
            m, s = t % 4, t // 4
            rows = slice(32 * m, 32 * m + K)
            c0 = s * SPAN
            lhsT = t_inp[rows, c0:c0 + P]
            rhs = t_inp[rows, c0 + P:c0 + SPAN]
            b = (t % 8) * 512
            out = ps[:, b:b + W]
            return lhsT, rhs, out

        def ps_group(k):
            """[P, 4, W] strided PSUM view of chunk k's 4 tiles."""
            c0 = (k % 2) * 2048
            return ps[:, c0:c0 + 2048].rearrange(
                "p (g f) -> p g f", f=512)[:, :, 0:W]

        @block.sync
        def _(sync):
            # quadrant m rows -> partitions 32m..32m+11; split each quadrant
            # into chunk-0-3 / chunk-4-7 halves so PE can start earlier
            for h in range(2):
                cols = slice(h * 4 * SPAN, (h + 1) * 4 * SPAN)
                for m in range(4):
                    sync.dma_start(
                        t_inp[32 * m:32 * m + K, cols],
                        inp[K * m:K * (m + 1), cols],
                    ).then_inc(dma_sem, 16)
            sync.wait_ge(dve_sem, NCHUNK)
            sync.dma_start(outd, acc[:]).then_inc(dma_sem, 16)

        @block.tensor
        def _(pe):
            pe.wait_ge(dma_sem, 64)
            for k in range(NCHUNK):
                if k == 4:
                    pe.wait_ge(dma_sem, 128)
                if k >= 2:
                    pe.wait_ge(dve_sem, k - 1)   # bank pair free
                last = None
                for m in range(4):
                    t = 4 * k + m
                    lhsT, rhs, out = tile_aps(t)
                    last = nc.tensor.matmul(
                        out, lhsT, rhs, start=True, stop=True,
                        tile_position=(32 * m, 0),
                    )
                last.then_inc(pe_sem, 1)

        @block.vector
        def _(vector):
            for k in range(NCHUNK):
                if k in chunks_b:
                    vector.wait_ge(act_sem, bidx[k] + 1)
                    src = bbuf[:, bidx[k] % 2, :].rearrange(
                        "p (g f) -> p g f", f=W)
                else:
                    vector.wait_ge(pe_sem, k + 1)
                    src = ps_group(k)
                nc.vector.tensor_reduce(
                    acc[:, 4 * k:4 * k + 4], src, axis=X, op=MIN,
                ).then_inc(dve_sem, 1)

        @block.scalar
        def _(scalar):
            for i, k in enumerate(sorted(chunks_b)):
                scalar.wait_ge(pe_sem, k + 1)
                if i >= 2:
                    # DVE finished reading bbuf slot i-2 once chunk
                    # sorted_b[i-2] (and everything before it) reduced
                    kprev = sorted(chunks_b)[i - 2]
                    scalar.wait_ge(dve_sem, kprev + 1)
                nc.scalar.activation(
                    bbuf[:, i % 2, :].rearrange("p (g f) -> p g f", f=W),
                    ps_group(k), COPY,
                ).then_inc(act_sem, 1)

    return nc


def _get_program():
    if "nc" not in _COMPILED:
        _COMPILED["nc"] = _build_program()
    return _COMPILED["nc"]


# ---------------- host-side prep ----------------

def _bf16(x):
    import ml_dtypes
    return np.asarray(x, dtype=ml_dtypes.bfloat16).astype(np.float64)


def _split(z):
    """z (f64) -> (hi, lo) bf16-representable f64 pair, hi+lo ~= z."""
    hi = _bf16(z)
    lo = _bf16(z - hi)
    return hi, lo


def _tile_order(pts):
    """Sort into 16 x-strips of 1024, y-sorted within each strip.

    Returns (order, strip_xlo, strip_xhi, strip_y) where order[t*128:(t+1)*128]
    is tile t; strip s = tiles [s*TPS, (s+1)*TPS).
    """
    n = pts.shape[0]
    per = n // NSTRIP
    ox = np.argsort(pts[:, 0], kind="stable")
    order = np.empty(n, dtype=np.int64)
    for s in range(NSTRIP):
        seg = ox[s * per:(s + 1) * per]
        oy = np.argsort(pts[seg, 1], kind="stable")
        order[s * per:(s + 1) * per] = seg[oy]
    xs = pts[order, 0]
    strip_xlo = np.array([xs[s * per:(s + 1) * per].min() for s in range(NSTRIP)])
    strip_xhi = np.array([xs[s * per:(s + 1) * per].max() for s in range(NSTRIP)])
    strip_y = pts[order, 1].reshape(NSTRIP, per)
    return order, strip_xlo, strip_xhi, strip_y


def _gather_candidates(box, opp_sorted, opp_xlo, opp_xhi, opp_y):
    """Indices (into opp sorted order) of points in the dilated box.

    Returns (idx, m_eff): all points NOT in idx are at Chebyshev distance
    > m_eff from the (undilated) box.
    """
    x0, x1, y0, y1 = box
    per = opp_y.shape[1]
    m = MARGIN
    while True:
        xlo, xhi, ylo, yhi = x0 - m, x1 + m, y0 - m, y1 + m
        runs = []
        for s in range(NSTRIP):
            if opp_xhi[s] < xlo or opp_xlo[s] > xhi:
                continue
            a = np.searchsorted(opp_y[s], ylo, side="left")
            b = np.searchsorted(opp_y[s], yhi, side="right")
            if b > a:
                runs.append(s * per + np.arange(a, b))
        idx = np.concatenate(runs) if runs else np.empty(0, dtype=np.int64)
        if idx.size:
            xv = opp_sorted[idx, 0]
            idx = idx[(xv >= xlo) & (xv <= xhi)]
        if idx.size <= W or m <= 0.5:
            break
        m *= 0.6     # overflow (never for uniform data): shrink margin
    if idx.size > W:
        idx = idx[:W]
        m = 0.0
    return idx, m


def _make_in_maps(p: np.ndarray, q: np.ndarray):
    """Tile both sides, gather windows, build device operands."""
    po, pxlo, pxhi, pyv = _tile_order(p)
    qo, qxlo, qxhi, qyv = _tile_order(q)
    ps_ = p[po].astype(np.float64)
    qs_ = q[qo].astype(np.float64)

    in_maps = []
    meta = []    # per core: list of (side, T, bound) per device tile
    for c in range(NCORES):
        arr = np.zeros((4 * K, 8 * SPAN), dtype=np.float64)
        tmeta = []
        for t in range(NDT):
            side = "p" if t < TPC else "q"
            T = 16 * c + (t if t < TPC else t - TPC)
            if side == "p":
                qry = ps_[T * P:(T + 1) * P]
                opp, oxlo, oxhi, oy = qs_, qxlo, qxhi, qyv
            else:
                qry = qs_[T * P:(T + 1) * P]
                opp, oxlo, oxhi, oy = ps_, pxlo, pxhi, pyv
            box = (qry[:, 0].min(), qry[:, 0].max(),
                   qry[:, 1].min(), qry[:, 1].max())
            idx, m_eff = _gather_candidates(box, opp, oxlo, oxhi, oy)
            if idx.size == 0:
                cand = np.zeros((W, 2))
                m_eff = -1.0     # force fallback for whole tile
            else:
                cand = opp[idx]
                if cand.shape[0] < W:
                    pad = np.broadcast_to(cand[0], (W - cand.shape[0], 2))
                    cand = np.concatenate([cand, pad], axis=0)
            cx = 0.5 * (box[0] + box[1])
            cy = 0.5 * (box[2] + box[3])
            uxh, uxl = _split(qry[:, 0] - cx)
            uyh, uyl = _split(qry[:, 1] - cy)
            su = (uxh + uxl) ** 2 + (uyh + uyl) ** 2
            sh, sl = _split(su)
            vxh, vxl = _split(cand[:, 0] - cx)
            vyh, vyl = _split(cand[:, 1] - cy)
            tv = (vxh + vxl) ** 2 + (vyh + vyl) ** 2
            th, tl = _split(tv)
            one = np.ones(P)
            onew = np.ones(W)
            lhsT = np.stack([uxh, uxh, uxl, uxl, uyh, uyh, uyl, uyl,
                             sh, sl, one, one])
            rhs = np.stack([-2 * vxh, -2 * vxl, -2 * vxh, -2 * vxl,
                            -2 * vyh, -2 * vyl, -2 * vyh, -2 * vyl,
                            onew, onew, th, tl])
            m4, s4 = t % 4, t // 4
            c0 = s4 * SPAN
            arr[K * m4:K * (m4 + 1), c0:c0 + P] = lhsT
            arr[K * m4:K * (m4 + 1), c0 + P:c0 + SPAN] = rhs
            tmeta.append((side, T, m_eff * m_eff))
        import ml_dtypes
        in_maps.append({"inp": arr.astype(ml_dtypes.bfloat16)})
        meta.append(tmeta)
    return in_maps, meta, po, qo, ps_, qs_


def kernel(img_render_points: np.ndarray, contour_points: np.ndarray) -> np.ndarray:
    # NOTE: do not enable jax_compilation_cache_dir here — loading this
    # program from the jax persistent cache produces executables that fail
    # with NRT_EXEC_UNIT_UNRECOVERABLE on the axon PJRT path.
    from concourse.bass_utils import run_bass_kernel_spmd

    p = np.asarray(img_render_points, dtype=np.float32).reshape(-1, 2)
    q = np.asarray(contour_points, dtype=np.float32)
    assert p.shape == (N, 2) and q.shape == (M, 2)

    in_maps, meta, po, qo, ps_, qs_ = _make_in_maps(p, q)

    nc = _get_program()
    res = run_bass_kernel_spmd(nc, in_maps, list(range(NCORES)))
    results = res.results

    # ---- certify + assemble ----
    min2_p = np.empty(N, dtype=np.float64)   # sorted-p order
    min2_q = np.empty(M, dtype=np.float64)   # sorted-q order
    bad_p, bad_q = [], []
    for c in range(NCORES):
        out = np.asarray(results[c]["out"], dtype=np.float64)  # [P, NDT]
        for t in range(NDT):
            side, T, bound = meta[c][t]
            v = np.maximum(out[:, t], 0.0)
            # numeric slack: matmul ~0.15, + bf16 cast for ACT-path chunks
            eta = 0.15 + (0.006 * v + 0.1 if (t // 4) in CHUNKS_B else 0.0)
            ok = v + eta <= bound
            dst = min2_p if side == "p" else min2_q
            dst[T * P:(T + 1) * P] = v
            fail = np.nonzero(~ok)[0]
            if fail.size:
                (bad_p if side == "p" else bad_q).append(T * P + fail)

    # ---- exact numpy fallback for any uncertified queries ----
    if bad_p:
        rows = np.concatenate(bad_p)
        d2 = ((ps_[rows, None, :] - qs_[None, :, :]) ** 2).sum(-1)
        min2_p[rows] = d2.min(axis=1)
    if bad_q:
        rows = np.concatenate(bad_q)
        d2 = ((qs_[rows, None, :] - ps_[None, :, :]) ** 2).sum(-1)
        min2_q[rows] = d2.min(axis=1)

    total = np.sqrt(min2_p).sum() + np.sqrt(min2_q).sum()
    return np.float32(total)


# revision 5
# speedup vs baseline: 7.0775x; 1.1836x over previous
"""Bidirectional chamfer loss on 8 Trainium2 NeuronCores.

Problem: N=16384 render points (128x128x2), M=16384 contour points (16384x2),
output = sum_i min_j ||p_i - q_j|| + sum_j min_i ||p_i - q_j||  (scalar f32).

Strategy (retrieval_knn, v3):
  - Host: 2D-tile both point sets (16 x-strips x 8 y-tiles = 128 tiles of 128
    points each side). For each query tile, gather the opposite-side points
    inside the tile's bbox dilated by MARGIN px (~209 expected for uniform
    data at MARGIN=6; W=320 budget, padded by repeating a real candidate so
    the min is unchanged). Points outside the dilated box are >MARGIN away
    from every query in the tile, so the windowed min equals the true min
    whenever the windowed min^2 < MARGIN^2 - slack; this is certified per
    query on the host, with an exact numpy fallback for any failure, so the
    kernel is correct for any input.
  - Device (per core, 16 p-tiles + 16 q-tiles = 32 tiles): the tensor engine
    computes each tile's [128 x 320] d2 block with a K=12 bf16 matmul.
    Coordinates are centered per tile and split hi/lo into bf16 pairs
    (Dekker-style), making the bf16 matmul accurate to ~0.05px^2 while
    running at 1 cycle/row (fp32 is 4). Tiles round-robin the 4 PE row
    quadrants so 4 matmuls pack concurrently in the array.
  - Reduce: DVE tensor_reduce(min) straight from PSUM, one segmented
    [128, 4, 320] reduce per 4-tile chunk. (Measured: every DVE reduce
    variant — any dtype, any layout, pool/max8 — runs at 1 elem/cycle/lane,
    so offload paths only add overhead; minimizing reduced elements is the
    only lever.)
  - The 4 quadrant input DMAs issue from 4 different engine queues (sync,
    gpsimd, vector, scalar) so they overlap instead of serializing.
  - Host: certify, sqrt + sum in float64, cast to float32.
"""

import numpy as np

# ---- hardcoded problem geometry (from the problem spec) ----
N = 16384            # render points (128*128)
M = 16384            # contour points
NCORES = 8
P = 128              # partitions / tile query count
NSTRIP = 16          # x-strips per side
TPS = 8              # y-tiles per strip
NTILE = NSTRIP * TPS              # 128 tiles per side
TPC = NTILE // NCORES             # 16 tiles per side per core
NDT = 2 * TPC                     # 32 device tiles per core
W = 320              # candidate window per tile
MARGIN = 8.0         # bbox dilation in px (auto-shrunk on overflow)
K = 12               # matmul contraction rows (hi/lo split form)
NCHUNK = NDT // 4    # 8 chunks of 4 quadrant-packed tiles
SPAN = P + W         # per-tile sbuf span: 128 lhsT cols + W rhs cols

_COMPILED = {}


def _build_program():
    """Build the SPMD bass program (same program for all 8 cores).

    Raw bass (not Tile): explicit Block bodies keep every semaphore wait a
    standalone instruction (walrus has 1 wait slot per instruction).
    """
    import concourse.bass as bass
    from concourse import mybir

    f32 = mybir.dt.float32
    bf16 = mybir.dt.bfloat16
    X = mybir.AxisListType.X
    MIN = mybir.AluOpType.min

    nc = bass.Bass("TRN2", target_bir_lowering=False, debug=False,
                   num_devices=NCORES)

    # dram input: 4 quadrant row-blocks of K=12 rows, 8 tiles x SPAN cols
    inp = nc.dram_tensor("inp", [4 * K, 8 * SPAN], bf16,
                         kind="ExternalInput").ap()
    outd = nc.dram_tensor("out", [P, NDT], f32, kind="ExternalOutput").ap()

    with (
        nc.sbuf_tensor([128, 8 * SPAN], bf16) as t_inp,
        nc.sbuf_tensor([P, NDT], f32) as acc,
        nc.psum_tensor([P, 4096], f32) as ps,
        nc.semaphore() as dma_sem,
        nc.semaphore() as pe_sem,
        nc.semaphore() as dve_sem,
        nc.Block() as block,
    ):
        def tile_aps(t):
            """(lhsT, rhs, psum_out) APs for device tile t."""
            m, s = t % 4, t // 4
            rows = slice(32 * m, 32 * m + K)
            c0 = s * SPAN
            lhsT = t_inp[rows, c0:c0 + P]
            rhs = t_inp[rows, c0 + P:c0 + SPAN]
            b = (t % 8) * 512
            out = ps[:, b:b + W]
            return lhsT, rhs, out

        def ps_group(k):
            """[P, 4, W] strided PSUM view of chunk k's 4 tiles."""
            c0 = (k % 2) * 2048
            return ps[:, c0:c0 + 2048].rearrange(
                "p (g f) -> p g f", f=512)[:, :, 0:W]

        def in_dma(eng, m):
            """quadrant m rows -> partitions 32m..32m+11."""
            eng.dma_start(
                t_inp[32 * m:32 * m + K, :],
                inp[K * m:K * (m + 1), :],
            ).then_inc(dma_sem, 16)

        @block.sync
        def _(sync):
            in_dma(sync, 0)
            in_dma(sync, 2)
            sync.wait_ge(dve_sem, NCHUNK // 2)
            sync.dma_start(outd[:, 0:NDT // 2],
                           acc[:, 0:NDT // 2]).then_inc(dma_sem, 16)
            sync.wait_ge(dve_sem, NCHUNK)
            sync.dma_start(outd[:, NDT // 2:],
                           acc[:, NDT // 2:]).then_inc(dma_sem, 16)

        @block.gpsimd
        def _(gp):
            in_dma(gp, 1)

        @block.tensor
        def _(pe):
            pe.wait_ge(dma_sem, 64)
            for k in range(NCHUNK):
                if k >= 2:
                    pe.wait_ge(dve_sem, k - 1)   # bank pair free
                last = None
                for m in range(4):
                    lhsT, rhs, out = tile_aps(4 * k + m)
                    last = nc.tensor.matmul(
                        out, lhsT, rhs, start=True, stop=True,
                        tile_position=(32 * m, 0),
                    )
                last.then_inc(pe_sem, 1)

        @block.vector
        def _(vector):
            for k in range(NCHUNK):
                vector.wait_ge(pe_sem, k + 1)
                nc.vector.tensor_reduce(
                    acc[:, 4 * k:4 * k + 4], ps_group(k), axis=X, op=MIN,
                ).then_inc(dve_sem, 1)

        @block.scalar
        def _(scalar):
            in_dma(scalar, 3)

    return nc


def _get_program():
    if "nc" not in _COMPILED:
        _COMPILED["nc"] = _build_program()
    return _COMPILED["nc"]


# ---------------- host-side prep ----------------

def _bf16(x):
    import ml_dtypes
    return np.asarray(x, dtype=ml_dtypes.bfloat16).astype(np.float64)


def _split(z):
    """z (f64) -> (hi, lo) bf16-representable f64 pair, hi+lo ~= z."""
    hi = _bf16(z)
    lo = _bf16(z - hi)
    return hi, lo


def _tile_order(pts):
    """Sort into 16 x-strips of 1024, y-sorted within each strip.

    Returns (order, strip_xlo, strip_xhi, strip_y) where order[t*128:(t+1)*128]
    is tile t; strip s = tiles [s*TPS, (s+1)*TPS).
    """
    n = pts.shape[0]
    per = n // NSTRIP
    ox = np.argsort(pts[:, 0], kind="stable")
    order = np.empty(n, dtype=np.int64)
    for s in range(NSTRIP):
        seg = ox[s * per:(s + 1) * per]
        oy = np.argsort(pts[seg, 1], kind="stable")
        order[s * per:(s + 1) * per] = seg[oy]
    xs = pts[order, 0]
    strip_xlo = np.array([xs[s * per:(s + 1) * per].min() for s in range(NSTRIP)])
    strip_xhi = np.array([xs[s * per:(s + 1) * per].max() for s in range(NSTRIP)])
    strip_y = pts[order, 1].reshape(NSTRIP, per)
    return order, strip_xlo, strip_xhi, strip_y


def _gather_candidates(box, opp_sorted, opp_xlo, opp_xhi, opp_y):
    """Indices (into opp sorted order) of points in the dilated box.

    Returns (idx, m_eff): all points NOT in idx are at Chebyshev distance
    > m_eff from the (undilated) box.
    """
    x0, x1, y0, y1 = box
    per = opp_y.shape[1]
    m = MARGIN
    while True:
        xlo, xhi, ylo, yhi = x0 - m, x1 + m, y0 - m, y1 + m
        runs = []
        for s in range(NSTRIP):
            if opp_xhi[s] < xlo or opp_xlo[s] > xhi:
                continue
            a = np.searchsorted(opp_y[s], ylo, side="left")
            b = np.searchsorted(opp_y[s], yhi, side="right")
            if b > a:
                runs.append(s * per + np.arange(a, b))
        idx = np.concatenate(runs) if runs else np.empty(0, dtype=np.int64)
        if idx.size:
            xv = opp_sorted[idx, 0]
            idx = idx[(xv >= xlo) & (xv <= xhi)]
        if idx.size <= W or m <= 0.5:
            break
        m *= 0.6     # overflow (never for uniform data): shrink margin
    if idx.size > W:
        idx = idx[:W]
        m = 0.0
    return idx, m


def _make_in_maps(p: np.ndarray, q: np.ndarray):
    """Tile both sides, gather windows, build device operands."""
    po, pxlo, pxhi, pyv = _tile_order(p)
    qo, qxlo, qxhi, qyv = _tile_order(q)
    ps_ = p[po].astype(np.float64)
    qs_ = q[qo].astype(np.float64)

    in_maps = []
    meta = []    # per core: list of (side, T, bound) per device tile
    for c in range(NCORES):
        arr = np.zeros((4 * K, 8 * SPAN), dtype=np.float64)
        tmeta = []
        for t in range(NDT):
            side = "p" if t < TPC else "q"
            T = 16 * c + (t if t < TPC else t - TPC)
            if side == "p":
                qry = ps_[T * P:(T + 1) * P]
                opp, oxlo, oxhi, oy = qs_, qxlo, qxhi, qyv
            else:
                qry = qs_[T * P:(T + 1) * P]
                opp, oxlo, oxhi, oy = ps_, pxlo, pxhi, pyv
            box = (qry[:, 0].min(), qry[:, 0].max(),
                   qry[:, 1].min(), qry[:, 1].max())
            idx, m_eff = _gather_candidates(box, opp, oxlo, oxhi, oy)
            if idx.size == 0:
                cand = np.zeros((W, 2))
                m_eff = -1.0     # force fallback for whole tile
            else:
                cand = opp[idx]
                if cand.shape[0] < W:
                    pad = np.broadcast_to(cand[0], (W - cand.shape[0], 2))
                    cand = np.concatenate([cand, pad], axis=0)
            cx = 0.5 * (box[0] + box[1])
            cy = 0.5 * (box[2] + box[3])
            uxh, uxl = _split(qry[:, 0] - cx)
            uyh, uyl = _split(qry[:, 1] - cy)
            su = (uxh + uxl) ** 2 + (uyh + uyl) ** 2
            sh, sl = _split(su)
            vxh, vxl = _split(cand[:, 0] - cx)
            vyh, vyl = _split(cand[:, 1] - cy)
            tv = (vxh + vxl) ** 2 + (vyh + vyl) ** 2
            th, tl = _split(tv)
            one = np.ones(P)
            onew = np.ones(W)
            lhsT = np.stack([uxh, uxh, uxl, uxl, uyh, uyh, uyl, uyl,
                             sh, sl, one, one])
            rhs = np.stack([-2 * vxh, -2 * vxl, -2 * vxh, -2 * vxl,
                            -2 * vyh, -2 * vyl, -2 * vyh, -2 * vyl,
                            onew, onew, th, tl])
            m4, s4 = t % 4, t // 4
            c0 = s4 * SPAN
            arr[K * m4:K * (m4 + 1), c0:c0 + P] = lhsT
            arr[K * m4:K * (m4 + 1), c0 + P:c0 + SPAN] = rhs
            tmeta.append((side, T, m_eff * m_eff))
        import ml_dtypes
        in_maps.append({"inp": arr.astype(ml_dtypes.bfloat16)})
        meta.append(tmeta)
    return in_maps, meta, po, qo, ps_, qs_


def kernel(img_render_points: np.ndarray, contour_points: np.ndarray) -> np.ndarray:
    # NOTE: do not enable jax_compilation_cache_dir here — loading this
    # program from the jax persistent cache produces executables that fail
    # with NRT_EXEC_UNIT_UNRECOVERABLE on the axon PJRT path.
    from concourse.bass_utils import run_bass_kernel_spmd

    p = np.asarray(img_render_points, dtype=np.float32).reshape(-1, 2)
    q = np.asarray(contour_points, dtype=np.float32)
    assert p.shape == (N, 2) and q.shape == (M, 2)

    in_maps, meta, po, qo, ps_, qs_ = _make_in_maps(p, q)

    nc = _get_program()
    res = run_bass_kernel_spmd(nc, in_maps, list(range(NCORES)))
    results = res.results

    # ---- certify + assemble ----
    min2_p = np.empty(N, dtype=np.float64)   # sorted-p order
    min2_q = np.empty(M, dtype=np.float64)   # sorted-q order
    bad_p, bad_q = [], []
    for c in range(NCORES):
        out = np.asarray(results[c]["out"], dtype=np.float64)  # [P, NDT]
        for t in range(NDT):
            side, T, bound = meta[c][t]
            v = np.maximum(out[:, t], 0.0)
            ok = v + 0.15 <= bound           # matmul numeric slack ~0.05px^2
            dst = min2_p if side == "p" else min2_q
            dst[T * P:(T + 1) * P] = v
            fail = np.nonzero(~ok)[0]
            if fail.size:
                (bad_p if side == "p" else bad_q).append(T * P + fail)

    # ---- exact numpy fallback for any uncertified queries ----
    if bad_p:
        rows = np.concatenate(bad_p)
        d2 = ((ps_[rows, None, :] - qs_[None, :, :]) ** 2).sum(-1)
        min2_p[rows] = d2.min(axis=1)
    if bad_q:
        rows = np.concatenate(bad_q)
        d2 = ((qs_[rows, None, :] - ps_[None, :, :]) ** 2).sum(-1)
        min2_q[rows] = d2.min(axis=1)

    total = np.sqrt(min2_p).sum() + np.sqrt(min2_q).sum()
    return np.float32(total)


# revision 12
# speedup vs baseline: 8.0055x; 1.1311x over previous
"""Bidirectional chamfer loss on 8 Trainium2 NeuronCores.

Problem: N=16384 render points (128x128x2), M=16384 contour points (16384x2),
output = sum_i min_j ||p_i - q_j|| + sum_j min_i ||p_i - q_j||  (scalar f32).

Strategy (retrieval_knn, v4):
  - Host: 2D-tile both point sets (16 x-strips x 16 y-tiles = 256 tiles of 64
    points each side, ~32x32px boxes). For each query tile, gather the
    opposite-side points inside the tile's bbox dilated by MARGIN=8px (~144
    expected for uniform data; W=224 budget, padded by repeating a real
    candidate so the min is unchanged). Points outside the dilated box are
    >MARGIN away from every query in the tile, so the windowed min equals
    the true min whenever the windowed min^2 < MARGIN^2 - slack; certified
    per query on the host, with an exact numpy fallback for any failure, so
    the kernel is correct for any input.
  - Device (per core, 64 query-tiles in 32 stacked PSUM slots): the tensor
    engine computes two [64 x 224] d2 blocks per PSUM bank — one at
    partitions 0-63 (tile_position col 0), one at 64-127 (col 64). K=10
    bf16 matmul: coordinates centered per tile, split hi/lo into bf16 pairs
    (Dekker-style, lo*lo cross terms dropped), accurate to ~0.05px^2 at
    1 cycle/row. Slots round-robin the 4 PE row quadrants so 8 matmuls
    pack concurrently.
  - Reduce: DVE tensor_reduce(min) straight from PSUM, one segmented
    [128, 4, 224] reduce per 4-slot chunk. (Measured: every DVE reduce
    variant runs at 1 elem/cycle/lane; minimizing reduced elements is the
    only lever — stacking halves the per-lane element count.)
  - Input DMAs: 8 transfers (4 quadrants x 2 halves) spread over the sync,
    gpsimd and scalar queues; PE starts after the first-half set.
  - Host: certify, sqrt + sum in float64, cast to float32.
"""

import numpy as np

# ---- hardcoded problem geometry (from the problem spec) ----
N = 16384            # render points (128*128)
M = 16384            # contour points
NCORES = 8
PT = 64              # queries per tile
NSTRIP = 16          # x-strips per side
TPS = 16             # y-tiles per strip
NTILE = NSTRIP * TPS              # 256 tiles per side
TPC = NTILE // NCORES             # 32 tiles per side per core
NDT = 2 * TPC                     # 64 device tiles per core
NSLOT = NDT // 2                  # 32 stacked psum slots
W = 224              # candidate window per tile
MARGIN = 8.0         # bbox dilation in px (auto-shrunk on overflow)
K = 10               # matmul contraction rows (hi/lo split form)
NCHUNK = NSLOT // 4  # 8 chunks of 4 quadrant-packed slots
SSPAN = 2 * (PT + W)              # per-slot sbuf span: 576

_COMPILED = {}


def _build_program():
    """Build the SPMD bass program (same program for all 8 cores).

    Raw bass (not Tile): explicit Block bodies keep every semaphore wait a
    standalone instruction (walrus has 1 wait slot per instruction).
    """
    import concourse.bass as bass
    from concourse import mybir

    f32 = mybir.dt.float32
    bf16 = mybir.dt.bfloat16
    X = mybir.AxisListType.X
    MIN = mybir.AluOpType.min

    nc = bass.Bass("TRN2", target_bir_lowering=False, debug=False,
                   num_devices=NCORES)

    # dram input: 4 quadrant row-blocks of K rows, 8 slots x SSPAN cols
    inp = nc.dram_tensor("inp", [4 * K, 8 * SSPAN], bf16,
                         kind="ExternalInput").ap()
    outd = nc.dram_tensor("out", [128, NSLOT], f32, kind="ExternalOutput").ap()

    with (
        nc.sbuf_tensor([128, 8 * SSPAN], bf16) as t_inp,
        nc.sbuf_tensor([128, NSLOT], f32) as acc,
        nc.psum_tensor([128, 4096], f32) as ps,
        nc.semaphore() as dma0_sem,
        nc.semaphore() as dma1_sem,
        nc.semaphore() as out_sem,
        nc.semaphore() as pe_sem,
        nc.semaphore() as dve_sem,
        nc.Block() as block,
    ):
        def slot_aps(j, half):
            """(lhsT, rhs, psum_out, tile_position) for slot j, half 0/1."""
            m, sq = j % 4, j // 4
            rows = slice(32 * m, 32 * m + K)
            c0 = sq * SSPAN + half * (PT + W)
            lhsT = t_inp[rows, c0:c0 + PT]
            rhs = t_inp[rows, c0 + PT:c0 + PT + W]
            b = (j % 8) * 512
            out = ps[64 * half:64 * half + 64, b:b + W]
            return lhsT, rhs, out, (32 * m, 64 * half)

        def ps_group(k):
            """[128, 4, W] strided PSUM view of chunk k's 4 slots."""
            c0 = (k % 2) * 2048
            return ps[:, c0:c0 + 2048].rearrange(
                "p (g f) -> p g f", f=512)[:, :, 0:W]

        def in_dma(eng, m, h):
            """quadrant m rows, half h -> partitions 32m..32m+K-1."""
            cols = slice(h * 4 * SSPAN, (h + 1) * 4 * SSPAN)
            eng.dma_start(
                t_inp[32 * m:32 * m + K, cols],
                inp[K * m:K * (m + 1), cols],
            ).then_inc(dma0_sem if h == 0 else dma1_sem, 16)

        @block.sync
        def _(sync):
            in_dma(sync, 0, 0)
            in_dma(sync, 0, 1)
            in_dma(sync, 3, 1)
            sync.wait_ge(dve_sem, NCHUNK // 2)
            sync.dma_start(outd[:, 0:NSLOT // 2],
                           acc[:, 0:NSLOT // 2]).then_inc(out_sem, 16)
            sync.wait_ge(dve_sem, NCHUNK)
            sync.dma_start(outd[:, NSLOT // 2:],
                           acc[:, NSLOT // 2:]).then_inc(out_sem, 16)

        @block.gpsimd
        def _(gp):
            in_dma(gp, 1, 0)
            in_dma(gp, 3, 0)
            in_dma(gp, 1, 1)

        @block.scalar
        def _(scalar):
            in_dma(scalar, 2, 0)
            in_dma(scalar, 2, 1)

        @block.tensor
        def _(pe):
            pe.wait_ge(dma0_sem, 64)     # all four half-0 transfers
            for k in range(NCHUNK):
                if k == 4:
                    pe.wait_ge(dma1_sem, 64)
                if k >= 2:
                    pe.wait_ge(dve_sem, k - 1)   # bank group free
                # all tops then all bottoms: spaces each row group's two
                # weight loads; each quadrant's LAST matmul bumps pe_sem so
                # the reduce can't run ahead of a straggling quadrant (PE
                # p-state ramp skews completion order on cold runs).
                for half in range(2):
                    for m in range(4):
                        lhsT, rhs, out, tp = slot_aps(4 * k + m, half)
                        mm = nc.tensor.matmul(
                            out, lhsT, rhs, start=True, stop=True,
                            tile_position=tp,
                        )
                        if half == 1:
                            mm.then_inc(pe_sem, 1)

        @block.vector
        def _(vector):
            for k in range(NCHUNK):
                vector.wait_ge(pe_sem, 4 * (k + 1))
                nc.vector.tensor_reduce(
                    acc[:, 4 * k:4 * k + 4], ps_group(k), axis=X, op=MIN,
                ).then_inc(dve_sem, 1)

    return nc


def _get_program():
    if "nc" not in _COMPILED:
        _COMPILED["nc"] = _build_program()
    return _COMPILED["nc"]


# ---------------- host-side prep ----------------

def _bf16(x):
    import ml_dtypes
    return np.asarray(x, dtype=ml_dtypes.bfloat16).astype(np.float64)


def _split(z):
    """z (f64) -> (hi, lo) bf16-representable f64 pair, hi+lo ~= z."""
    hi = _bf16(z)
    lo = _bf16(z - hi)
    return hi, lo


def _tile_order(pts):
    """Sort into 16 x-strips of 1024, y-sorted within each strip.

    Returns (order, strip_xlo, strip_xhi, strip_y); tile t (64 points) =
    order[t*PT:(t+1)*PT]; strip s = tiles [s*TPS, (s+1)*TPS).
    """
    n = pts.shape[0]
    per = n // NSTRIP
    ox = np.argsort(pts[:, 0], kind="stable")
    order = np.empty(n, dtype=np.int64)
    for s in range(NSTRIP):
        seg = ox[s * per:(s + 1) * per]
        oy = np.argsort(pts[seg, 1], kind="stable")
        order[s * per:(s + 1) * per] = seg[oy]
    xs = pts[order, 0]
    strip_xlo = np.array([xs[s * per:(s + 1) * per].min() for s in range(NSTRIP)])
    strip_xhi = np.array([xs[s * per:(s + 1) * per].max() for s in range(NSTRIP)])
    strip_y = pts[order, 1].reshape(NSTRIP, per)
    return order, strip_xlo, strip_xhi, strip_y


def _gather_candidates(box, opp_sorted, opp_xlo, opp_xhi, opp_y):
    """Indices (into opp sorted order) of points in the dilated box.

    Returns (idx, m_eff): all points NOT in idx are at Chebyshev distance
    > m_eff from the (undilated) box.
    """
    x0, x1, y0, y1 = box
    per = opp_y.shape[1]
    m = MARGIN
    while True:
        xlo, xhi, ylo, yhi = x0 - m, x1 + m, y0 - m, y1 + m
        runs = []
        for s in range(NSTRIP):
            if opp_xhi[s] < xlo or opp_xlo[s] > xhi:
                continue
            a = np.searchsorted(opp_y[s], ylo, side="left")
            b = np.searchsorted(opp_y[s], yhi, side="right")
            if b > a:
                runs.append(s * per + np.arange(a, b))
        idx = np.concatenate(runs) if runs else np.empty(0, dtype=np.int64)
        if idx.size:
            xv = opp_sorted[idx, 0]
            idx = idx[(xv >= xlo) & (xv <= xhi)]
        if idx.size <= W or m <= 0.5:
            break
        m *= 0.6     # overflow (never for uniform data): shrink margin
    if idx.size > W:
        idx = idx[:W]
        m = 0.0
    return idx, m


def _operands(qry, cand):
    """K=10 hi/lo bf16 rows: (lhsT [K, PT], rhs [K, W]) as float64."""
    cx = 0.5 * (qry[:, 0].min() + qry[:, 0].max())
    cy = 0.5 * (qry[:, 1].min() + qry[:, 1].max())
    uxh, uxl = _split(qry[:, 0] - cx)
    uyh, uyl = _split(qry[:, 1] - cy)
    su = (uxh + uxl) ** 2 + (uyh + uyl) ** 2
    sh, sl = _split(su)
    vxh, vxl = _split(cand[:, 0] - cx)
    vyh, vyl = _split(cand[:, 1] - cy)
    tv = (vxh + vxl) ** 2 + (vyh + vyl) ** 2
    th, tl = _split(tv)
    one = np.ones(qry.shape[0])
    onew = np.ones(cand.shape[0])
    lhsT = np.stack([uxh, uxh, uxl, uyh, uyh, uyl, sh, sl, one, one])
    rhs = np.stack([-2 * vxh, -2 * vxl, -2 * vxh,
                    -2 * vyh, -2 * vyl, -2 * vyh,
                    onew, onew, th, tl])
    return lhsT, rhs


def _make_in_maps(p: np.ndarray, q: np.ndarray):
    """Tile both sides, gather windows, build device operands."""
    po, pxlo, pxhi, pyv = _tile_order(p)
    qo, qxlo, qxhi, qyv = _tile_order(q)
    ps_ = p[po].astype(np.float64)
    qs_ = q[qo].astype(np.float64)

    in_maps = []
    meta = []    # per core: list of (side, T, bound) per device tile
    for c in range(NCORES):
        arr = np.zeros((4 * K, 8 * SSPAN), dtype=np.float64)
        tmeta = []
        for t in range(NDT):
            side = "p" if t < TPC else "q"
            T = TPC * c + (t if t < TPC else t - TPC)
            if side == "p":
                qry = ps_[T * PT:(T + 1) * PT]
                opp, oxlo, oxhi, oy = qs_, qxlo, qxhi, qyv
            else:
                qry = qs_[T * PT:(T + 1) * PT]
                opp, oxlo, oxhi, oy = ps_, pxlo, pxhi, pyv
            box = (qry[:, 0].min(), qry[:, 0].max(),
                   qry[:, 1].min(), qry[:, 1].max())
            idx, m_eff = _gather_candidates(box, opp, oxlo, oxhi, oy)
            if idx.size == 0:
                cand = np.zeros((W, 2))
                m_eff = -1.0     # force fallback for whole tile
            else:
                cand = opp[idx]
                if cand.shape[0] < W:
                    pad = np.broadcast_to(cand[0], (W - cand.shape[0], 2))
                    cand = np.concatenate([cand, pad], axis=0)
            lhsT, rhs = _operands(qry, cand)
            j, half = t // 2, t % 2
            m4, sq = j % 4, j // 4
            c0 = sq * SSPAN + half * (PT + W)
            arr[K * m4:K * (m4 + 1), c0:c0 + PT] = lhsT
            arr[K * m4:K * (m4 + 1), c0 + PT:c0 + PT + W] = rhs
            tmeta.append((side, T, m_eff * m_eff))
        import ml_dtypes
        in_maps.append({"inp": arr.astype(ml_dtypes.bfloat16)})
        meta.append(tmeta)
    return in_maps, meta, po, qo, ps_, qs_


def kernel(img_render_points: np.ndarray, contour_points: np.ndarray) -> np.ndarray:
    # NOTE: do not enable jax_compilation_cache_dir here — loading this
    # program from the jax persistent cache produces executables that fail
    # with NRT_EXEC_UNIT_UNRECOVERABLE on the axon PJRT path.
    from concourse.bass_utils import run_bass_kernel_spmd

    p = np.asarray(img_render_points, dtype=np.float32).reshape(-1, 2)
    q = np.asarray(contour_points, dtype=np.float32)
    assert p.shape == (N, 2) and q.shape == (M, 2)

    in_maps, meta, po, qo, ps_, qs_ = _make_in_maps(p, q)

    nc = _get_program()
    res = run_bass_kernel_spmd(nc, in_maps, list(range(NCORES)))
    results = res.results

    # ---- certify + assemble ----
    min2_p = np.empty(N, dtype=np.float64)   # sorted-p order
    min2_q = np.empty(M, dtype=np.float64)   # sorted-q order
    bad_p, bad_q = [], []
    for c in range(NCORES):
        out = np.asarray(results[c]["out"], dtype=np.float64)  # [128, NSLOT]
        for t in range(NDT):
            side, T, bound = meta[c][t]
            j, half = t // 2, t % 2
            v = np.maximum(out[64 * half:64 * half + 64, j], 0.0)
            ok = v + 0.15 <= bound           # matmul numeric slack ~0.05px^2
            dst = min2_p if side == "p" else min2_q
            dst[T * PT:(T + 1) * PT] = v
            fail = np.nonzero(~ok)[0]
            if fail.size:
                (bad_p if side == "p" else bad_q).append(T * PT + fail)

    # ---- exact numpy fallback for any uncertified queries ----
    if bad_p:
        rows = np.concatenate(bad_p)
        d2 = ((ps_[rows, None, :] - qs_[None, :, :]) ** 2).sum(-1)
        min2_p[rows] = d2.min(axis=1)
    if bad_q:
        rows = np.concatenate(bad_q)
        d2 = ((qs_[rows, None, :] - ps_[None, :, :]) ** 2).sum(-1)
        min2_q[rows] = d2.min(axis=1)

    total = np.sqrt(min2_p).sum() + np.sqrt(min2_q).sum()
    return np.float32(total)
